# revision 8
# baseline (speedup 1.0000x reference)
"""Multi-head attention on 8 TRN2 NeuronCores.

Reference computation (per batch b):
  q = x @ w_q;  k, v = split(x @ w_kv);  per head: softmax(q k^T / 8) v
  out = ctx @ w_out + b_out

Sharding: core c handles batch b = c // 2 and head-half hh = c % 2
(8 of 16 heads). Each core computes a partial out^T (its 8 heads'
contribution, transposed); the host sums the two partials per batch,
adds the bias and transposes back.

Per-core kernel (everything transposed, feature-major; all matmul free
dims 512 = one psum bank):
  xT  [1024, 2048] (dl, s) bf16, host-pretransposed.
  QT/KT per head-pair p: [128, 2048] = (2 heads x 64 dh, s).
  V natural [s-tile, 8*64] per s-tile (no ones column).
  Heads are processed as two COUPLES (pairs 0,1 then 2,3). Per
  (couple, qchunk 512, ki):
    scores^T: one [128,1024] psum tile per pair = 2 row-tiled K=64
      matmuls (heads run CONCURRENTLY in PE row halves).
    P^T = Exp(scale * scores^T) on ACT (psum -> sbuf bf16), N=1024.
    ctx^T: per pair 2 col-tiled M=64 matmuls (heads in PE column
      halves, psum partition halves) -- CONCURRENT, accumulated over ki.
    denominators: one 4-way col-tiled slot of M=4 ones-matmuls
      (4 heads of the couple at col positions 0/32/64/96), accumulated
      over ki into one held psum bank.
  Chunk end: ctx psum -> ct sbuf (raw); recip of denom stripes; the
  broadcast/multiply normalize chain is deferred one chunk so its
  cross-engine latency hides under the next chunk's compute.
  QK projections, the V projection and the out projection are
  interleaved into the attention stream as 8-matmul [128,512]-psum
  groups (one drained per ki step) to keep the PE dense. Output DMAs
  issue from the ACT DGE queue so the SP queue reaches the next
  iteration's input DMAs early (loop-boundary overlap).
"""

import numpy as np
import ml_dtypes

import concourse.bacc as bacc
import concourse.tile as tile
import concourse.mybir as mybir
from concourse.bass_utils import run_bass_kernel_spmd

bf16 = ml_dtypes.bfloat16
FP32 = mybir.dt.float32
BF16 = mybir.dt.bfloat16
EXP = mybir.ActivationFunctionType.Exp

B, S, DL = 4, 2048, 1024
H, DH = 16, 64          # global heads
NH = 8                  # heads per core
HD = NH * DH            # 512 feature cols per core
NPAIR = NH // 2         # 4 head pairs
KT = DL // 128          # 8 k-tiles over d_latent
ST = S // 128           # 16 tiles over sequence
NQC = S // 512          # 4 q-chunks of 512
SCALE = 1.0 / np.sqrt(DH)

N_CORES = 8
ABLATE = set()   # timing-only ablations: 'den', 'out', 'proj'


def _build(reps: int = 1, loop: int = 0):
    nc = bacc.Bacc(None, target_bir_lowering=False)

    xT = nc.dram_tensor("xT", [DL, S], BF16, kind="ExternalInput")
    wq = nc.dram_tensor("wq", [DL, HD], BF16, kind="ExternalInput")
    wk = nc.dram_tensor("wk", [DL, HD], BF16, kind="ExternalInput")
    wv = nc.dram_tensor("wv", [DL, HD], BF16, kind="ExternalInput")
    wo = nc.dram_tensor("wo", [HD, DL], BF16, kind="ExternalInput")
    out = nc.dram_tensor("out", [DL, S], FP32, kind="ExternalOutput")

    with tile.TileContext(nc) as tc:
        with (
            tc.tile_pool(name="persist", bufs=1) as pp,
            tc.tile_pool(name="pt", bufs=4) as ptp,
            tc.tile_pool(name="small", bufs=4) as smp,
            tc.tile_pool(name="outsb", bufs=2) as osp,
            tc.tile_pool(name="psS", bufs=2, space="PSUM") as psS,
            tc.tile_pool(name="psH", bufs=1, space="PSUM") as psH,
            tc.tile_pool(name="psP", bufs=1, space="PSUM") as psP,
        ):
            pools = (pp, ptp, smp, osp, psS, psH, psP)
            if loop:
                with tc.For_i(0, loop, 1):
                    _body(nc, tc, pools, xT, wq, wk, wv, wo, out)
            else:
                for _ in range(reps):
                    _body(nc, tc, pools, xT, wq, wk, wv, wo, out)
    nc.compile()
    return nc


def _body(nc, tc, pools, xT, wq, wk, wv, wo, out):
    pp, ptp, smp, osp, psS, psH, psP = pools

    # ---- persistent tiles ----
    xt = [pp.tile([128, S], BF16, tag=f"xt{k}", name=f"xt{k}") for k in range(KT)]
    wq_sb = [pp.tile([128, HD], BF16, tag=f"wq{k}", name=f"wq{k}") for k in range(KT)]
    wk_sb = [pp.tile([128, HD], BF16, tag=f"wk{k}", name=f"wk{k}") for k in range(KT)]
    wv_sb = [pp.tile([128, HD], BF16, tag=f"wv{k}", name=f"wv{k}") for k in range(KT)]
    wo_sb = [pp.tile([128, DL], BF16, tag=f"wo{t}", name=f"wo{t}") for t in range(NPAIR)]
    qt = [pp.tile([128, S], BF16, tag=f"qt{p}", name=f"qt{p}") for p in range(NPAIR)]
    kt_ = [pp.tile([128, S], BF16, tag=f"kt{p}", name=f"kt{p}") for p in range(NPAIR)]
    vt = [pp.tile([128, HD], BF16, tag=f"vt{m}", name=f"vt{m}") for m in range(ST)]
    ct = [pp.tile([128, S], BF16, tag=f"ct{t}", name=f"ct{t}") for t in range(NPAIR)]
    ones = pp.tile([128, 32], BF16, tag="ones")

    # ---- input DMAs (SP queue) ----
    for k in range(KT):
        nc.sync.dma_start(xt[k][:, :], xT[k * 128:(k + 1) * 128, :])
        nc.sync.dma_start(wq_sb[k][:, :], wq[k * 128:(k + 1) * 128, :])
        nc.sync.dma_start(wk_sb[k][:, :], wk[k * 128:(k + 1) * 128, :])
        nc.sync.dma_start(wv_sb[k][:, :], wv[k * 128:(k + 1) * 128, :])
    for t in range(NPAIR):
        nc.sync.dma_start(wo_sb[t][:, :], wo[t * 128:(t + 1) * 128, :])
    nc.vector.memset(ones[:, :], 1.0)

    # ---- interleavable PE work units (8-matmul groups, 1 psum bank) ----
    def vproj_group(m):
        if 'proj' in ABLATE:
            nc.sync.dma_start(vt[m][:, :], xT[0:128, 0:HD])
            return
        ps = psP.tile([128, 512], FP32, tag="pj", name=f"vp{m}")
        for k in range(KT):
            nc.tensor.matmul(ps[:, :],
                             xt[k][:, m * 128:(m + 1) * 128],
                             wv_sb[k][:, :],
                             start=(k == 0), stop=(k == KT - 1))
        nc.vector.tensor_copy(vt[m][:, :], ps[:, :])

    def qkproj_group(p, dst, w_sb, ch):
        rsl = slice(ch * 512, ch * 512 + 512)
        if 'proj' in ABLATE:
            nc.sync.dma_start(dst[:, rsl], xT[0:128, rsl])
            return
        ps = psP.tile([128, 512], FP32, tag="pj", name=f"qk{p}c{ch}")
        for k in range(KT):
            nc.tensor.matmul(ps[:, :],
                             w_sb[k][:, p * 128:(p + 1) * 128],
                             xt[k][:, rsl],
                             start=(k == 0), stop=(k == KT - 1))
        nc.vector.tensor_copy(dst[:, rsl], ps[:, :])

    def outproj_group(mt, qc):
        if 'out' in ABLATE:
            return
        msl = slice(mt * 128, (mt + 1) * 128)
        qsl = slice(qc * 512, qc * 512 + 512)
        ps = psP.tile([128, 512], FP32, tag="pj", name=f"op{mt}q{qc}")
        for t in range(NPAIR):
            nc.tensor.matmul(ps[:, :], wo_sb[t][:, msl], ct[t][:, qsl],
                             start=(t == 0), stop=(t == NPAIR - 1))
        ob = osp.tile([128, 512], FP32, tag="ob", name="ob")
        nc.vector.tensor_copy(ob[:, :], ps[:, :])
        nc.scalar.dma_start(out[msl, qsl], ob[:, :])

    pending = []

    def drain(n=1):
        for _ in range(min(n, len(pending))):
            pending.pop(0)()

    # prologue: what couple 0's first steps need (before step 0)
    for p in (0, 1):
        qkproj_group(p, kt_[p], wk_sb, 0)
        qkproj_group(p, qt[p], wq_sb, 0)
    for m in range(6):
        vproj_group(m)
    for p in (0, 1):
        qkproj_group(p, qt[p], wq_sb, 1)

    # work drained one unit per ki step, earliest-deadline-first.
    # qch0 steps 0..15: kt chunks before their first ki use, vt before ctx.
    for p in (0, 1):
        pending.append((lambda p=p: qkproj_group(p, kt_[p], wk_sb, 1)))
    pending.append((lambda: vproj_group(6)))
    for p in (0, 1):
        pending.append((lambda p=p: qkproj_group(p, kt_[p], wk_sb, 2)))
    for m in (7, 8, 9):
        pending.append((lambda m=m: vproj_group(m)))
    for p in (0, 1):
        pending.append((lambda p=p: qkproj_group(p, kt_[p], wk_sb, 3)))
    for m in range(10, ST):
        pending.append((lambda m=m: vproj_group(m)))
    # qch1+ steps: remaining couple-0 qt chunks, then couple 1's QK proj
    for p in (0, 1):
        pending.append((lambda p=p: qkproj_group(p, qt[p], wq_sb, 2)))
    for p in (2, 3):
        pending.append((lambda p=p: qkproj_group(p, kt_[p], wk_sb, 0)))
        pending.append((lambda p=p: qkproj_group(p, qt[p], wq_sb, 0)))
    for p in (2, 3):
        pending.append((lambda p=p: qkproj_group(p, kt_[p], wk_sb, 1)))
    for p in (0, 1):
        pending.append((lambda p=p: qkproj_group(p, qt[p], wq_sb, 3)))
    for ch in (2, 3):
        for p in (2, 3):
            pending.append((lambda p=p, ch=ch: qkproj_group(p, kt_[p], wk_sb, ch)))
    for ch in (1, 2, 3):
        for p in (2, 3):
            pending.append((lambda p=p, ch=ch: qkproj_group(p, qt[p], wq_sb, ch)))

    # deferred-normalize: (pair, qsl, rsrc) multiplied one chunk later
    norm_pend = []

    def flush_norm():
        while norm_pend:
            p_, qsl_, rsrc_ = norm_pend.pop(0)
            csl = ct[p_][:, qsl_]
            nc.vector.tensor_mul(csl, csl, rsrc_[:, :])

    for c in range(2):
        p0, p1 = 2 * c, 2 * c + 1

        for qch in range(NQC):
            qsl = slice(qch * 512, qch * 512 + 512)
            ctxA = psH.tile([128, 512], FP32, tag="ctxA", name="ctxA")
            ctxB = psH.tile([128, 512], FP32, tag="ctxB", name="ctxB")
            den = psH.tile([128, 512], FP32, tag="den", name="den")
            if 'den' in ABLATE:
                nc.vector.memset(den[:, :], 1.0)
            pend_ctx = None

            for ki in range(ST):
                ksl = slice(ki * 128, (ki + 1) * 128)
                scA = psS.tile([128, 1024], FP32, tag="sc", name="scA")
                nc.tensor.matmul(scA[:, 0:512], kt_[p0][0:64, ksl],
                                 qt[p0][0:64, qsl], start=True, stop=True,
                                 tile_position=(0, 0))
                nc.tensor.matmul(scA[:, 512:1024], kt_[p0][64:128, ksl],
                                 qt[p0][64:128, qsl], start=True, stop=True,
                                 tile_position=(64, 0))
                ptA = ptp.tile([128, 1024], BF16, tag="ptA", name="ptA")
                nc.scalar.activation(ptA[:, :], scA[:, :], EXP, scale=SCALE)

                scB = psS.tile([128, 1024], FP32, tag="sc", name="scB")
                nc.tensor.matmul(scB[:, 0:512], kt_[p1][0:64, ksl],
                                 qt[p1][0:64, qsl], start=True, stop=True,
                                 tile_position=(0, 0))
                nc.tensor.matmul(scB[:, 512:1024], kt_[p1][64:128, ksl],
                                 qt[p1][64:128, qsl], start=True, stop=True,
                                 tile_position=(64, 0))
                ptB = ptp.tile([128, 1024], BF16, tag="ptB", name="ptB")
                nc.scalar.activation(ptB[:, :], scB[:, :], EXP, scale=SCALE)

                def emit_ctx(kj, pa, pb):
                    st, sp = (kj == 0), (kj == ST - 1)
                    for pair_ps, pt_, pidx in ((ctxA, pa, p0), (ctxB, pb, p1)):
                        for hi in range(2):
                            lhl = (2 * pidx % NH + hi) * DH
                            nc.tensor.matmul(
                                pair_ps[hi * 64:hi * 64 + 64, :],
                                vt[kj][:, lhl:lhl + DH],
                                pt_[:, hi * 512:hi * 512 + 512],
                                start=st, stop=sp,
                                skip_group_check=True,
                                tile_position=(0, hi * 64))
                    for j, pt_ in enumerate((pa, pa, pb, pb) if 'den' not in ABLATE else ()):
                        nc.tensor.matmul(den[32 * j:32 * j + 32, :],
                                         ones[:, :],
                                         pt_[:, (j % 2) * 512:(j % 2) * 512 + 512],
                                         start=st, stop=sp,
                                         skip_group_check=True,
                                         tile_position=(0, 32 * j))

                if pend_ctx is not None:
                    emit_ctx(*pend_ctx)
                pend_ctx = (ki, ptA, ptB)
                if ki == 1:
                    flush_norm()
                drain(1)
            emit_ctx(*pend_ctx)

            # ---- chunk end: raw ctx -> sbuf, recip denom stripes ----
            rsA = smp.tile([128, 512], BF16, tag="rsA", name="rsA")
            rsB = smp.tile([128, 512], BF16, tag="rsB", name="rsB")
            with nc.allow_low_precision(reason="softmax denom recip in bf16"):
                nc.vector.reciprocal(rsA[0:32, :], den[0:32, :])
                nc.vector.reciprocal(rsA[64:96, :], den[32:64, :])
                nc.vector.reciprocal(rsB[0:32, :], den[64:96, :])
                nc.vector.reciprocal(rsB[64:96, :], den[96:128, :])
            nc.vector.tensor_copy(ct[p0][:, qsl], ctxA[:, :])
            nc.vector.tensor_copy(ct[p1][:, qsl], ctxB[:, :])
            # replicate recip quadrants into the sibling quadrant of each head
            nc.sync.dma_start(rsA[32:64, :], rsA[0:32, :])
            nc.sync.dma_start(rsA[96:128, :], rsA[64:96, :])
            nc.sync.dma_start(rsB[32:64, :], rsB[0:32, :])
            nc.sync.dma_start(rsB[96:128, :], rsB[64:96, :])
            norm_pend.append((p0, qsl, rsA))
            norm_pend.append((p1, qsl, rsB))

            # couple 1: queue out-proj for the newest fully-normalized chunk
            if c == 1 and qch >= 1:
                for mt in range(KT):
                    pending.append(
                        (lambda mt=mt, qc=qch - 1: outproj_group(mt, qc)))

    flush_norm()
    drain(len(pending))
    # epilogue: out-proj for the last q-chunk
    for mt in range(KT):
        outproj_group(mt, NQC - 1)


_NC_CACHE = {}


def _get_nc(reps: int = 1):
    if reps not in _NC_CACHE:
        _NC_CACHE[reps] = _build(reps)
    return _NC_CACHE[reps]


def shard_inputs(x, w_q, w_kv, w_out):
    """Full inputs -> per-core in_maps (host-side layout prep)."""
    ins = []
    for c in range(N_CORES):
        b, hh = c // 2, c % 2
        fsl = slice(hh * HD, (hh + 1) * HD)
        ins.append({
            "xT": np.ascontiguousarray(x[b].T).astype(bf16),
            "wq": np.ascontiguousarray(w_q[:, fsl]).astype(bf16),
            "wk": np.ascontiguousarray(w_kv[:, fsl]).astype(bf16),
            "wv": np.ascontiguousarray(w_kv[:, H * DH:][:, fsl]).astype(bf16),
            "wo": np.ascontiguousarray(w_out[fsl, :]).astype(bf16),
        })
    return ins


def unshard_output(results, b_out):
    out = np.empty((B, S, DL), np.float32)
    for b in range(B):
        acc = results[2 * b]["out"] + results[2 * b + 1]["out"]   # [DL, S]
        out[b] = acc.T + b_out
    return out


def kernel(x, w_q, w_kv, w_out, b_out):
    nc = _get_nc()
    ins = shard_inputs(x, w_q, w_kv, w_out)
    res = run_bass_kernel_spmd(nc, ins, core_ids=list(range(N_CORES)))
    return unshard_output(res.results, b_out)


# revision 10
# speedup vs baseline: 1.0722x; 1.0722x over previous
"""Multi-head attention on 8 TRN2 NeuronCores.

Reference computation (per batch b):
  q = x @ w_q;  k, v = split(x @ w_kv);  per head: softmax(q k^T / 8) v
  out = ctx @ w_out + b_out

Sharding: core c handles batch b = c // 2 and head-half hh = c % 2
(8 of 16 heads). Each core computes a partial out^T (its 8 heads'
contribution, transposed); the host sums the two partials per batch,
adds the bias and transposes back.

Per-core kernel (everything transposed, feature-major):
  xT  [1024, 2048] (dl, s) bf16, host-pretransposed.
  QT/KT per head-pair p: [128, 2048] = (2 heads x 64 dh, s).
  V natural [s-tile, 8 x (64+4)] with a ones column block per head ->
    the ctx^T matmul (lhsT = V|1) also produces the softmax denominator
    as psum rows 64:68 for free.
  Per (pair, qchunk 512, ki):
    scores^T: one [128,1024] psum tile = 2 row-tiled K=64 matmuls
      (both heads CONCURRENT in PE row halves).
    P^T = Exp(scale * scores^T) on ACT (psum -> sbuf bf16), N=1024.
    ctx^T: 2 matmuls M=68 (V|1), accumulated over ki into a held
      [68, 1024] psum tile (head h in columns h*512..).
  Chunk end: raw ctx + denominator rows -> SBUF; the broadcast/recip/
  multiply normalize chain is deferred one chunk so its cross-engine
  latency hides under the next chunk's compute.
  The V projection, QK projections for later pairs, and the out
  projection are interleaved into the attention stream as small
  [128,512]-psum groups (one drained per ki step, earliest-deadline-
  first) to keep the PE dense. Output DMAs issue from the ACT DGE
  queue so the SP queue reaches the next iteration's input DMAs early.
"""

import numpy as np
import ml_dtypes

import concourse.bacc as bacc
import concourse.tile as tile
import concourse.mybir as mybir
from concourse.bass_utils import run_bass_kernel_spmd

bf16 = ml_dtypes.bfloat16
FP32 = mybir.dt.float32
BF16 = mybir.dt.bfloat16
EXP = mybir.ActivationFunctionType.Exp

B, S, DL = 4, 2048, 1024
H, DH = 16, 64          # global heads
NH = 8                  # heads per core
HD = NH * DH            # 512 feature cols per core
DHP = DH + 4            # head slot width in vt (V plus ones block)
NPAIR = NH // 2         # 4 head pairs
KT = DL // 128          # 8 k-tiles over d_latent
ST = S // 128           # 16 tiles over sequence
NQC = S // 512          # 4 q-chunks of 512
SCALE = 1.0 / np.sqrt(DH)

N_CORES = 8
ABLATE = set()   # timing-only ablations: 'out', 'proj'


def _build(reps: int = 1, loop: int = 0):
    nc = bacc.Bacc(None, target_bir_lowering=False)

    xT = nc.dram_tensor("xT", [DL, S], BF16, kind="ExternalInput")
    wq = nc.dram_tensor("wq", [DL, HD], BF16, kind="ExternalInput")
    wk = nc.dram_tensor("wk", [DL, HD], BF16, kind="ExternalInput")
    wv = nc.dram_tensor("wv", [DL, HD], BF16, kind="ExternalInput")
    wo = nc.dram_tensor("wo", [HD, DL], BF16, kind="ExternalInput")
    out = nc.dram_tensor("out", [DL, S], FP32, kind="ExternalOutput")

    with tile.TileContext(nc) as tc:
        with (
            tc.tile_pool(name="persist", bufs=1) as pp,
            tc.tile_pool(name="pt", bufs=4) as ptp,
            tc.tile_pool(name="small", bufs=2) as smp,
            tc.tile_pool(name="outsb", bufs=2) as osp,
            tc.tile_pool(name="psS", bufs=2, space="PSUM") as psS,
            tc.tile_pool(name="psH", bufs=1, space="PSUM") as psH,
            tc.tile_pool(name="psP", bufs=2, space="PSUM") as psP,
        ):
            pools = (pp, ptp, smp, osp, psS, psH, psP)
            if loop:
                with tc.For_i(0, loop, 1):
                    _body(nc, tc, pools, xT, wq, wk, wv, wo, out)
            else:
                for _ in range(reps):
                    _body(nc, tc, pools, xT, wq, wk, wv, wo, out)
    nc.compile()
    return nc


def _body(nc, tc, pools, xT, wq, wk, wv, wo, out):
    pp, ptp, smp, osp, psS, psH, psP = pools

    # ---- persistent tiles ----
    xt = [pp.tile([128, S], BF16, tag=f"xt{k}", name=f"xt{k}") for k in range(KT)]
    wq_sb = [pp.tile([128, HD], BF16, tag=f"wq{k}", name=f"wq{k}") for k in range(KT)]
    wk_sb = [pp.tile([128, HD], BF16, tag=f"wk{k}", name=f"wk{k}") for k in range(KT)]
    wv_sb = [pp.tile([128, HD], BF16, tag=f"wv{k}", name=f"wv{k}") for k in range(KT)]
    wo_sb = [pp.tile([128, DL], BF16, tag=f"wo{t}", name=f"wo{t}") for t in range(NPAIR)]
    qt = [pp.tile([128, S], BF16, tag=f"qt{p}", name=f"qt{p}") for p in range(NPAIR)]
    kt_ = [pp.tile([128, S], BF16, tag=f"kt{p}", name=f"kt{p}") for p in range(NPAIR)]
    vt = [pp.tile([128, NH * DHP], BF16, tag=f"vt{m}", name=f"vt{m}") for m in range(ST)]
    ct = [pp.tile([128, S], BF16, tag=f"ct{t}", name=f"ct{t}") for t in range(NPAIR)]

    # ---- input DMAs (SP queue) ----
    for k in range(KT):
        nc.sync.dma_start(xt[k][:, :], xT[k * 128:(k + 1) * 128, :])
        nc.sync.dma_start(wq_sb[k][:, :], wq[k * 128:(k + 1) * 128, :])
        nc.sync.dma_start(wk_sb[k][:, :], wk[k * 128:(k + 1) * 128, :])
        nc.sync.dma_start(wv_sb[k][:, :], wv[k * 128:(k + 1) * 128, :])
    for t in range(NPAIR):
        nc.sync.dma_start(wo_sb[t][:, :], wo[t * 128:(t + 1) * 128, :])

    # ---- interleavable PE work units ([128,512]-psum groups) ----
    def vproj_group(m):
        if 'proj' in ABLATE:
            nc.sync.dma_start(vt[m][:, :], xT[0:128, 0:NH * DHP])
            return
        ps = psP.tile([128, 512], FP32, tag="pj", name=f"vp{m}")
        for k in range(KT):
            nc.tensor.matmul(ps[:, :],
                             xt[k][:, m * 128:(m + 1) * 128],
                             wv_sb[k][:, :],
                             start=(k == 0), stop=(k == KT - 1))
        v3 = vt[m][:, :].rearrange("p (h c) -> p h c", c=DHP)
        nc.vector.tensor_copy(v3[:, :, 0:DH],
                              ps[:, :].rearrange("p (h c) -> p h c", h=NH))
        nc.vector.memset(v3[:, :, DH:DHP], 1.0)

    def qkproj_group(p, dst, w_sb, ch):
        rsl = slice(ch * 512, ch * 512 + 512)
        if 'proj' in ABLATE:
            nc.sync.dma_start(dst[:, rsl], xT[0:128, rsl])
            return
        ps = psP.tile([128, 512], FP32, tag="pj", name=f"qk{p}c{ch}")
        for k in range(KT):
            nc.tensor.matmul(ps[:, :],
                             w_sb[k][:, p * 128:(p + 1) * 128],
                             xt[k][:, rsl],
                             start=(k == 0), stop=(k == KT - 1))
        nc.vector.tensor_copy(dst[:, rsl], ps[:, :])

    def outproj_group(mt, qc):
        if 'out' in ABLATE:
            return
        msl = slice(mt * 128, (mt + 1) * 128)
        qsl = slice(qc * 512, qc * 512 + 512)
        ps = psP.tile([128, 512], FP32, tag="pj", name=f"op{mt}q{qc}")
        for t in range(NPAIR):
            nc.tensor.matmul(ps[:, :], wo_sb[t][:, msl], ct[t][:, qsl],
                             start=(t == 0), stop=(t == NPAIR - 1))
        ob = osp.tile([128, 512], FP32, tag="ob", name="ob")
        nc.vector.tensor_copy(ob[:, :], ps[:, :])
        nc.scalar.dma_start(out[msl, qsl], ob[:, :])

    pending = []

    def drain(n=1):
        for _ in range(min(n, len(pending))):
            pending.pop(0)()

    # prologue: what pair 0's first steps need (before step 0)
    qkproj_group(0, kt_[0], wk_sb, 0)
    qkproj_group(0, qt[0], wq_sb, 0)
    for m in range(4):
        vproj_group(m)

    # earliest-deadline-first work queue, one unit per ki step.
    # pair0 qch0 (steps 0..15): kt chunks before first use, vt before ctx.
    pending.append((lambda: qkproj_group(0, kt_[0], wk_sb, 1)))
    pending.append((lambda: vproj_group(4)))
    pending.append((lambda: vproj_group(5)))
    pending.append((lambda: qkproj_group(0, kt_[0], wk_sb, 2)))
    pending.append((lambda: vproj_group(6)))
    pending.append((lambda: vproj_group(7)))
    pending.append((lambda: vproj_group(8)))
    pending.append((lambda: qkproj_group(0, kt_[0], wk_sb, 3)))
    for m in (9, 10, 11, 12, 13, 14, 15):
        pending.append((lambda m=m: vproj_group(m)))
    pending.append((lambda: qkproj_group(0, qt[0], wq_sb, 1)))
    # pair0 qch1+ (steps 16+)
    pending.append((lambda: qkproj_group(0, qt[0], wq_sb, 2)))
    pending.append((lambda: qkproj_group(0, qt[0], wq_sb, 3)))
    # later pairs' projections (pair p starts at step 64p)
    for p in (1, 2, 3):
        for ch in range(NQC):
            pending.append((lambda p=p, ch=ch: qkproj_group(p, kt_[p], wk_sb, ch)))
        for ch in range(NQC):
            pending.append((lambda p=p, ch=ch: qkproj_group(p, qt[p], wq_sb, ch)))

    # deferred-normalize: (pair, qsl, rsrc) multiplied one chunk later
    norm_pend = []
    BCAST_MASK = [0] * 32

    def flush_norm():
        while norm_pend:
            p_, qsl_, rsrc_ = norm_pend.pop(0)
            rdst = smp.tile([128, 1024], BF16, tag="rdst", name="rdst")
            nc.vector.stream_shuffle(rdst[:, :], rsrc_[:, :], BCAST_MASK)
            for hi_ in range(2):
                psl = slice(hi_ * 64, (hi_ + 1) * 64)
                csl = ct[p_][psl, qsl_]
                nc.vector.tensor_mul(csl, csl,
                                     rdst[psl, hi_ * 512:hi_ * 512 + 512])

    for p in range(NPAIR):
        for qch in range(NQC):
            qsl = slice(qch * 512, qch * 512 + 512)
            rsrc = smp.tile([128, 1024], BF16, tag="rsrc", name="rsrc")
            nc.vector.memset(rsrc[:, :], 1.0)
            ctxp = psH.tile([DHP, 1024], FP32, tag="ctx", name="ctxp")
            ctx1 = ctxp[:, 0:512]
            ctx2 = ctxp[:, 512:1024]
            pend_ctx = None

            for ki in range(ST):
                ksl = slice(ki * 128, (ki + 1) * 128)
                sc = psS.tile([128, 1024], FP32, tag="sc", name="sc")
                nc.tensor.matmul(sc[:, 0:512], kt_[p][0:64, ksl],
                                 qt[p][0:64, qsl], start=True, stop=True,
                                 tile_position=(0, 0))
                nc.tensor.matmul(sc[:, 512:1024], kt_[p][64:128, ksl],
                                 qt[p][64:128, qsl], start=True, stop=True,
                                 tile_position=(64, 0))
                pt1 = ptp.tile([128, 1024], BF16, tag="pt", name="pt1")
                nc.scalar.activation(pt1[:, :], sc[:, :], EXP, scale=SCALE)

                def emit_ctx(kj, ptj):
                    for hi, ctx in enumerate((ctx1, ctx2)):
                        lh = 2 * p + hi
                        nc.tensor.matmul(ctx[:, :],
                                         vt[kj][:, lh * DHP:lh * DHP + DHP],
                                         ptj[:, hi * 512:hi * 512 + 512],
                                         start=(kj == 0), stop=(kj == ST - 1))

                if pend_ctx is not None:
                    emit_ctx(*pend_ctx)
                pend_ctx = (ki, pt1)
                if ki == 1:
                    flush_norm()
                drain(1)
            emit_ctx(*pend_ctx)

            # chunk end: raw ctx + denominator rows -> SBUF, free psum
            for hi, ctx in enumerate((ctx1, ctx2)):
                with nc.allow_low_precision(reason="softmax denom recip in bf16"):
                    nc.vector.reciprocal(rsrc[0:4, hi * 512:hi * 512 + 512],
                                         ctx[DH:DHP, :])
                nc.vector.tensor_copy(ct[p][hi * 64:(hi + 1) * 64, qsl],
                                      ctx[0:DH, :])
            # replicate the recip rows into every 32-partition quadrant
            for q_ in (32, 64, 96):
                nc.sync.dma_start(rsrc[q_:q_ + 4, :], rsrc[0:4, :])
            norm_pend.append((p, qsl, rsrc))

            # pair 3: queue out-proj for the newest fully-normalized chunk
            if p == 3 and qch >= 1:
                for mt in range(KT):
                    pending.append(
                        (lambda mt=mt, qc=qch - 1: outproj_group(mt, qc)))

    flush_norm()
    drain(len(pending))
    # epilogue: out-proj for the last q-chunk
    for mt in range(KT):
        outproj_group(mt, NQC - 1)


_NC_CACHE = {}


def _get_nc(reps: int = 1):
    if reps not in _NC_CACHE:
        _NC_CACHE[reps] = _build(reps)
    return _NC_CACHE[reps]


def shard_inputs(x, w_q, w_kv, w_out):
    """Full inputs -> per-core in_maps (host-side layout prep)."""
    ins = []
    for c in range(N_CORES):
        b, hh = c // 2, c % 2
        fsl = slice(hh * HD, (hh + 1) * HD)
        ins.append({
            "xT": np.ascontiguousarray(x[b].T).astype(bf16),
            "wq": np.ascontiguousarray(w_q[:, fsl]).astype(bf16),
            "wk": np.ascontiguousarray(w_kv[:, fsl]).astype(bf16),
            "wv": np.ascontiguousarray(w_kv[:, H * DH:][:, fsl]).astype(bf16),
            "wo": np.ascontiguousarray(w_out[fsl, :]).astype(bf16),
        })
    return ins


def unshard_output(results, b_out):
    out = np.empty((B, S, DL), np.float32)
    for b in range(B):
        acc = results[2 * b]["out"] + results[2 * b + 1]["out"]   # [DL, S]
        out[b] = acc.T + b_out
    return out


def kernel(x, w_q, w_kv, w_out, b_out):
    nc = _get_nc()
    ins = shard_inputs(x, w_q, w_kv, w_out)
    res = run_bass_kernel_spmd(nc, ins, core_ids=list(range(N_CORES)))
    return unshard_output(res.results, b_out)


# revision 14
# speedup vs baseline: 1.1180x; 1.0428x over previous
"""Multi-head attention on 8 TRN2 NeuronCores.

Reference computation (per batch b):
  q = x @ w_q;  k, v = split(x @ w_kv);  per head: softmax(q k^T / 8) v
  out = ctx @ w_out + b_out

Sharding: core c handles batch b = c // 2 and head-half hh = c % 2
(8 of 16 heads). Per-core work is a perfectly balanced 1/8 of total
MACs. Each core computes a partial out^T (its 8 heads' contribution,
transposed); the host sums the two partials per batch, adds the bias
and transposes back.

Per-core kernel layout (everything transposed, feature-major — this
avoids all on-chip transposes):
  xT  [1024, 2048]  (dl, s)      bf16, host-pretransposed
  QT/KT per head-pair p: [128, 2048] = (2 heads x 64 dh, s)
  V natural [s, dh] with a ones column appended per head -> the
    ctx^T matmul (lhsT = V|1) also produces the softmax denominator
    as psum row 64 for free.
  scores^T tile (s_k=128, q-chunk 1024) = paired K=64 matmuls via PE
    row tiling (two heads concurrently in array rows 0-63 / 64-127)
  P^T = Exp(scale * scores^T) on the scalar engine (psum -> sbuf bf16)
  ctx^T accumulated over 16 k-tiles; normalized by 1/denom which is
    broadcast across partitions with a K=1 ones matmul + DVE recip.
  out^T [1024, 2048] = w_out^T-slices @ ctx^T
"""

import numpy as np
import ml_dtypes

import concourse.bacc as bacc
import concourse.tile as tile
import concourse.mybir as mybir
from concourse.bass_utils import run_bass_kernel_spmd

bf16 = ml_dtypes.bfloat16
FP32 = mybir.dt.float32
BF16 = mybir.dt.bfloat16
EXP = mybir.ActivationFunctionType.Exp

B, S, DL = 4, 2048, 1024
H, DH = 16, 64          # global heads
NH = 8                  # heads per core
HD = NH * DH            # 512 feature cols per core
NPAIR = NH // 2         # 4 head pairs
KT = DL // 128          # 8 k-tiles over d_latent
ST = S // 128           # 16 tiles over sequence
QCH = 1024              # q-chunk (free dim of scores^T tiles)
NQC = S // QCH          # 2 q-chunks
SCALE = 1.0 / np.sqrt(DH)

N_CORES = 8


def _build(reps: int = 1, loop: int = 0, ablate=()):
    nc = bacc.Bacc(None, target_bir_lowering=False)

    xT = nc.dram_tensor("xT", [DL, S], BF16, kind="ExternalInput")
    wq = nc.dram_tensor("wq", [DL, HD], BF16, kind="ExternalInput")
    wk = nc.dram_tensor("wk", [DL, HD], BF16, kind="ExternalInput")
    wv = nc.dram_tensor("wv", [DL, HD], BF16, kind="ExternalInput")
    wo = nc.dram_tensor("wo", [HD, DL], BF16, kind="ExternalInput")
    out = nc.dram_tensor("out", [DL, S], FP32, kind="ExternalOutput")

    with tile.TileContext(nc) as tc:
        with (
            tc.tile_pool(name="persist", bufs=1) as pp,
            tc.tile_pool(name="pt", bufs=4) as ptp,
            tc.tile_pool(name="small", bufs=2) as smp,
            tc.tile_pool(name="outsb", bufs=2) as osp,
            tc.tile_pool(name="psA", bufs=3, space="PSUM") as psA,
            tc.tile_pool(name="psB", bufs=1, space="PSUM") as psB,
        ):
            if loop:
                with tc.For_i(0, loop, 1):
                    _body(nc, tc, pp, ptp, smp, osp, psA, psB,
                          xT, wq, wk, wv, wo, out, ablate)
            else:
                for _ in range(reps):
                    _body(nc, tc, pp, ptp, smp, osp, psA, psB,
                          xT, wq, wk, wv, wo, out, ablate)
    nc.compile()
    return nc


def _body(nc, tc, pp, ptp, smp, osp, psA, psB, xT, wq, wk, wv, wo, out, ablate=()):
    # ---- persistent tiles (tag-keyed; reused across reps) ----
    xt = [pp.tile([128, S], BF16, tag=f"xt{k}", name=f"xt{k}") for k in range(KT)]
    wq_sb = [pp.tile([128, HD], BF16, tag=f"wq{k}", name=f"wq{k}") for k in range(KT)]
    wk_sb = [pp.tile([128, HD], BF16, tag=f"wk{k}", name=f"wk{k}") for k in range(KT)]
    wv_sb = [pp.tile([128, HD], BF16, tag=f"wv{k}", name=f"wv{k}") for k in range(KT)]
    wo_sb = [pp.tile([128, DL], BF16, tag=f"wo{t}", name=f"wo{t}") for t in range(NPAIR)]
    qt = [pp.tile([128, S], BF16, tag=f"qt{p}", name=f"qt{p}") for p in range(NPAIR)]
    kt_ = [pp.tile([128, S], BF16, tag=f"kt{p}", name=f"kt{p}") for p in range(NPAIR)]
    vt = [pp.tile([128, NH * (DH + 4)], BF16, tag=f"vt{m}", name=f"vt{m}") for m in range(ST)]
    ct = [pp.tile([128, S], BF16, tag=f"ct{t}", name=f"ct{t}") for t in range(NPAIR)]

    # ---- input DMAs ----
    for k in range(KT):
        nc.sync.dma_start(xt[k][:, :], xT[k * 128:(k + 1) * 128, :])
        nc.sync.dma_start(wq_sb[k][:, :], wq[k * 128:(k + 1) * 128, :])
        nc.sync.dma_start(wk_sb[k][:, :], wk[k * 128:(k + 1) * 128, :])
        nc.sync.dma_start(wv_sb[k][:, :], wv[k * 128:(k + 1) * 128, :])
    for t in range(NPAIR):
        nc.sync.dma_start(wo_sb[t][:, :], wo[t * 128:(t + 1) * 128, :])
    if 'proj' in ablate:
        for p_ in range(NPAIR):
            nc.sync.dma_start(qt[p_][:, :], xT[p_ * 128:(p_ + 1) * 128, :])
            nc.sync.dma_start(kt_[p_][:, :], xT[p_ * 128:(p_ + 1) * 128, :])
    if 'vproj' in ablate:
        for m_ in range(ST):
            nc.sync.dma_start(vt[m_][:, :], xT[(m_ % 8) * 128:(m_ % 8) * 128 + 128, 0:NH * (DH + 4)])
    if 'attn' in ablate:
        for t_ in range(NPAIR):
            nc.sync.dma_start(ct[t_][:, :], xT[t_ * 128:(t_ + 1) * 128, :])

    # ---- V projection (natural layout [s, hd], ones col per head) ----
    for m in range(ST if 'vproj' not in ablate else 0):
        ps = psA.tile([128, QCH], FP32, tag="sc")
        for k in range(KT):
            nc.tensor.matmul(ps[:, 0:HD],
                             xt[k][:, m * 128:(m + 1) * 128],
                             wv_sb[k][:, :],
                             start=(k == 0), stop=(k == KT - 1))
        v3 = vt[m][:, :].rearrange("p (h c) -> p h c", c=DH + 4)
        nc.vector.tensor_copy(v3[:, :, 0:DH],
                              ps[:, 0:HD].rearrange("p (h c) -> p h c", h=NH))
        nc.vector.memset(v3[:, :, DH:DH + 4], 1.0)

    # deferred-normalize machinery: at each chunk end the raw ctx and its
    # denominator row are copied to SBUF (releasing psum immediately); the
    # bcast/recip/in-place-mul chain is emitted one chunk later so its
    # cross-engine latency hides under the next chunk's compute.
    norm_pend = []
    BCAST_MASK = [0] * 32

    def flush_norm():
        while norm_pend:
            p_, qsl_, rsrc_ = norm_pend.pop(0)
            rdst = smp.tile([128, QCH], BF16, tag="rdst")
            nc.vector.stream_shuffle(rdst[:, :], rsrc_[:, :], BCAST_MASK)
            for hi_ in range(2):
                psl = slice(hi_ * 64, (hi_ + 1) * 64)
                csl = ct[p_][psl, qsl_]
                nc.vector.tensor_mul(csl, csl,
                                     rdst[psl, hi_ * 512:hi_ * 512 + 512])

    # Q^T/K^T projection emitters, one [128,1024] psum group per call;
    # interleaved into the attention stream of the previous pair.
    def proj_group(p_, dst, w_sb, nch):
        ps = psA.tile([128, QCH], FP32, tag="sc", name="proj")
        for half in range(2):
            nsl = slice(half * 512, half * 512 + 512)
            rsl = slice(nch * QCH + half * 512, nch * QCH + half * 512 + 512)
            for k in range(KT):
                nc.tensor.matmul(ps[:, nsl],
                                 w_sb[k][:, p_ * 128:(p_ + 1) * 128],
                                 xt[k][:, rsl],
                                 start=(k == 0), stop=(k == KT - 1))
        nc.vector.tensor_copy(dst[:, nch * QCH:(nch + 1) * QCH], ps[:, :])

    def proj_groups_for(p_):
        if 'proj' in ablate or p_ >= NPAIR:
            return []
        return [(p_, dst, w_sb, nch)
                for dst, w_sb in ((qt[p_], wq_sb), (kt_[p_], wk_sb))
                for nch in range(NQC)]

    for g in proj_groups_for(0):
        proj_group(*g)

    for p in range(NPAIR):
        pending_proj = proj_groups_for(p + 1)

        # ---- attention for the pair's two heads ----
        # QCH=512 per head; both heads' score tiles share one [128, 1024]
        # psum tile -> ONE exp per k-step covers both heads.
        for qch in range(0 if 'attn' in ablate else 4):
            qsl = slice(qch * 512, (qch + 1) * 512)
            rsrc = smp.tile([128, QCH], BF16, tag="rsrc")
            nc.vector.memset(rsrc[:, :], 1.0)
            if 'ctx' not in ablate:
                ctxp = psB.tile([DH + 4, QCH], FP32, tag="ctx", name="ctxp")
                ctx1 = ctxp[:, 0:512]
                ctx2 = ctxp[:, 512:1024]
            pend = []
            for ki in range(ST):
                sc = psA.tile([128, QCH], FP32, tag="sc", name="sc")
                ksl = slice(ki * 128, (ki + 1) * 128)
                nc.tensor.matmul(sc[:, 0:512], kt_[p][0:64, ksl],
                                 qt[p][0:64, qsl], start=True, stop=True,
                                 tile_position=(0, 0))
                nc.tensor.matmul(sc[:, 512:1024], kt_[p][64:128, ksl],
                                 qt[p][64:128, qsl], start=True, stop=True,
                                 tile_position=(64, 0))
                pt1 = ptp.tile([128, QCH], BF16, tag="pt", name="pt1")
                nc.scalar.activation(pt1[:, :], sc[:, :], EXP, scale=SCALE)

                def emit_ctx(kj, ptj):
                    for hi, ctx in enumerate((ctx1, ctx2)):
                        lh = 2 * p + hi
                        nc.tensor.matmul(ctx[:, :],
                                         vt[kj][:, lh * 68:lh * 68 + 68],
                                         ptj[:, hi * 512:hi * 512 + 512],
                                         start=(kj == 0), stop=(kj == ST - 1))
                if 'ctx' not in ablate:
                    # defer ctx TWO steps behind its exp producer so the PE
                    # never waits on the ACT engine finishing the exp
                    if len(pend) >= 2:
                        emit_ctx(*pend.pop(0))
                    pend.append((ki, pt1))
                if ki == 1:
                    flush_norm()
            if 'ctx' not in ablate:
                for e in pend:
                    emit_ctx(*e)
            # chunk end: raw ctx + denominator row -> SBUF, free psum
            if 'ctx' not in ablate and 'norm' not in ablate:
                for hi, ctx in enumerate((ctx1, ctx2)):
                    with nc.allow_low_precision(reason="softmax denom recip in bf16"):
                        nc.vector.reciprocal(rsrc[0:4, hi * 512:hi * 512 + 512],
                                             ctx[DH:DH + 4, :])
                    nc.vector.tensor_copy(ct[p][hi * 64:(hi + 1) * 64, qsl],
                                          ctx[0:DH, :])
                # replicate the recip rows into every 32-partition quadrant
                # (stream_shuffle only shuffles within quadrants)
                for q_ in (32, 64, 96):
                    nc.sync.dma_start(rsrc[q_:q_ + 4, :], rsrc[0:4, :])
                if 'chain' not in ablate:
                    norm_pend.append((p, qsl, rsrc))
            # interleave one projection group for the next pair
            if pending_proj:
                proj_group(*pending_proj.pop(0))

        for g in pending_proj:
            proj_group(*g)

    flush_norm()

    # ---- out^T projection ----
    for mt in range(KT if 'out' not in ablate else 0):
        msl = slice(mt * 128, (mt + 1) * 128)
        for nch in range(2):
            ob = osp.tile([128, QCH], FP32, tag="ob")
            ps = psA.tile([128, QCH], FP32, tag="sc")
            for half in range(2):
                nsl = slice(half * 512, half * 512 + 512)
                rsl = slice(nch * QCH + half * 512, nch * QCH + half * 512 + 512)
                for t in range(NPAIR):
                    nc.tensor.matmul(ps[:, nsl],
                                     wo_sb[t][:, msl],
                                     ct[t][:, rsl],
                                     start=(t == 0), stop=(t == NPAIR - 1))
            nc.vector.tensor_copy(ob[:, :], ps[:, :])
            # out DMA on the ACT DGE queue: the SP queue then reaches the
            # next loop iteration's input DMAs early (boundary overlap)
            nc.scalar.dma_start(out[msl, nch * QCH:(nch + 1) * QCH], ob[:, :])


_NC_CACHE = {}


def _get_nc(reps: int = 1):
    if reps not in _NC_CACHE:
        _NC_CACHE[reps] = _build(reps)
    return _NC_CACHE[reps]


def shard_inputs(x, w_q, w_kv, w_out):
    """Full inputs -> per-core in_maps (host-side layout prep)."""
    ins = []
    for c in range(N_CORES):
        b, hh = c // 2, c % 2
        fsl = slice(hh * HD, (hh + 1) * HD)
        ins.append({
            "xT": np.ascontiguousarray(x[b].T).astype(bf16),
            "wq": np.ascontiguousarray(w_q[:, fsl]).astype(bf16),
            "wk": np.ascontiguousarray(w_kv[:, fsl]).astype(bf16),
            "wv": np.ascontiguousarray(w_kv[:, H * DH:][:, fsl]).astype(bf16),
            "wo": np.ascontiguousarray(w_out[fsl, :]).astype(bf16),
        })
    return ins


def unshard_output(results, b_out):
    out = np.empty((B, S, DL), np.float32)
    for b in range(B):
        acc = results[2 * b]["out"] + results[2 * b + 1]["out"]   # [DL, S]
        out[b] = acc.T + b_out
    return out


def kernel(x, w_q, w_kv, w_out, b_out):
    nc = _get_nc()
    ins = shard_inputs(x, w_q, w_kv, w_out)
    res = run_bass_kernel_spmd(nc, ins, core_ids=list(range(N_CORES)))
    return unshard_output(res.results, b_out)

